# revision 23
# baseline (speedup 1.0000x reference)
"""Trainium2 Bass kernel for nn_AttentionHead (B=2, T=2048, C=2048, H=16 heads, D=128).

Sharding: tensor-parallel over heads - 2 heads per NeuronCore (8 cores).
Each core computes qkv for its heads, RoPE, causal softmax attention, and a
partial c_proj contribution; the host sums the 8 partial outputs.

v8 vs v2-baseline (414us -> ~350us measured):
  - All matmuls bf16.  (fp8/DoubleRow was tried and reverted: softmax
    averaging shrinks signal and quantization noise at the same rate, so
    single-pass fp8 anywhere in the qkv path costs ~2-3e-2 relmax -- over
    the 2e-2 gate.  Full hi/lo compensation needs 3 fp8 passes = 1.5x bf16
    time.  gpsimd den-accumulation was also tried and reverted: gpsimd
    tensor ops run at ~55 G elem/s, 1.2us per [128,512] add.)
  - Few, large DMAs: the ~0.6us-per-dma_start sequencer cost was starving
    the Scalar (70us) and Sync (120us) engines in v2.  Startup transfers
    are spread across the three DMA-capable queues (sync/scalar/gpsimd,
    ~50GB/s each) in consumption order; x blocks prefetched one block
    ahead on two queues.
  - stage_a in colgroup phases (Qtop/Qbot/Ktop/Kbot) with [128,512] moving
    operands: fewer, larger matmuls and fewer evictions than the v2
    half-block layout.
  - sg pool 3x[128,512] with per-chunk exp (finer PSUM release for the
    S -> exp chain), pool_a 3 bufs; c_proj evictions on DVE except the
    drain-phase blocks which alternate Scalar/Vector; partial outputs
    written bf16 (halves out-DMA bytes; host sums in f32).
"""

import sys

sys.path.insert(0, "/opt/trn_rl_repo")

import ml_dtypes
import numpy as np

import concourse.mybir as mybir
import concourse.tile as tile
from concourse import bacc
from concourse.bass_utils import run_bass_kernel_spmd

F32 = mybir.dt.float32
BF16 = mybir.dt.bfloat16
NP_BF16 = np.dtype(ml_dtypes.bfloat16)

B, T, C, H, D = 2, 2048, 2048, 16, 128
NC_CORES = 8
HPC = H // NC_CORES            # heads per core = 2
BT = B * T                     # 4096
NKT = C // 128                 # 16 contraction tiles (128-chan)
TBS = 512                      # query block size
NTB_B = T // TBS               # 4 query blocks per batch
INV_SQRT_D = 1.0 / float(np.sqrt(D))

_CACHE = {}


def _build_program():
    nc = bacc.Bacc(None)

    # xb: per block, [128 chan-lo, ktile, 512 tok] bf16
    xbd = nc.dram_tensor("xb", [B * NTB_B, 128, NKT, TBS], BF16, kind="ExternalInput")
    wqk = nc.dram_tensor("wqk", [NKT, 128, 4 * 128], BF16, kind="ExternalInput")
    wv = nc.dram_tensor("wv", [NKT, 128, HPC * D], BF16, kind="ExternalInput")
    wp = nc.dram_tensor("wp", [HPC, C // 512, 128, 512], BF16, kind="ExternalInput")
    cs = nc.dram_tensor("cs", [4, 128, 512], BF16, kind="ExternalInput")
    sn = nc.dram_tensor("sn", [4, 128, 512], BF16, kind="ExternalInput")
    ones_f = nc.dram_tensor("ones_f", [128, 128], BF16, kind="ExternalInput")
    out_d = nc.dram_tensor("out", [BT, C], BF16, kind="ExternalOutput")

    NU = B * NTB_B              # 8 query blocks across both batches

    with tile.TileContext(nc) as tc:
        with (
            tc.tile_pool(name="const", bufs=1) as constp,
            tc.tile_pool(name="xp", bufs=2) as xp,         # x block tiles
            tc.tile_pool(name="qk", bufs=2) as qkp,        # QH/KH per batch
            tc.tile_pool(name="vp", bufs=2) as vpool,      # VH per batch
            tc.tile_pool(name="yp", bufs=2) as ypool,      # yT per batch
            tc.tile_pool(name="rin", bufs=3) as rinp,      # rope inputs (tops/bots)
            tc.tile_pool(name="tmp", bufs=4) as tmpp,      # rope temporaries
            tc.tile_pool(name="rt", bufs=3) as rtp,        # rope outputs
            tc.tile_pool(name="pp", bufs=3) as ppool,      # P stripes
            tc.tile_pool(name="rc", bufs=2) as rcp,        # reciprocal out
            tc.tile_pool(name="outp", bufs=2) as outp,     # c_proj staging
            tc.tile_pool(name="ps_a", bufs=3, space="PSUM") as pool_a,
            tc.tile_pool(name="ps_sg", bufs=3, space="PSUM") as pool_sg,
            tc.tile_pool(name="ps_dv", bufs=2, space="PSUM") as pool_dv,
        ):
            # ---- constants ----
            # Startup is DMA-latency-bound: ~3.5MB (wqk + x-block-0 + wv +
            # rope tables) must land before stage_a(0) can finish, and a
            # single DMA queue moves only ~50GB/s.  Spread the startup
            # transfers across the three DMA-capable queues (sync, scalar,
            # gpsimd), interleaved in k-tile consumption order.
            wqk_s = constp.tile([128, NKT, 4 * 128], BF16, tag="wqk")
            wv_s = constp.tile([128, NKT, HPC * D], BF16, tag="wv")
            cs_s = constp.tile([128, T], BF16, tag="cs")
            sn_s = constp.tile([128, T], BF16, tag="sn")
            ones_s = constp.tile([128, 128], BF16, tag="ones")
            wp_s = constp.tile([128, HPC, C], BF16, tag="wp")
            xbt = [None] * NU
            xbt[0] = xp.tile([128, NKT, TBS], BF16, tag="xb", name="xbt0")

            def wqk_chunk(eng, ks):
                eng.dma_start(wqk_s[:, ks, :], wqk[ks].rearrange("k p m -> p k m"))

            def x0_chunk(eng, ks):
                eng.dma_start(xbt[0][:, ks, :], xbd[0][:, ks, :])

            engs = (nc.sync, nc.scalar, nc.gpsimd)
            qmap = [0, 1, 2] * 4 + [0, 1, 2, 1]   # k-tile -> queue, k-order
            for k in range(NKT):
                eng = engs[qmap[k]]
                wqk_chunk(eng, slice(k, k + 1))
                x0_chunk(eng, slice(k, k + 1))
                if k in (9, 11, 13, 15):
                    qv = (k - 9) // 2 * 4
                    engs[(k + 1) % 3].dma_start(
                        wv_s[:, qv : qv + 4, :],
                        wv[qv : qv + 4].rearrange("k p t -> p k t"),
                    )
            nc.scalar.dma_start(
                cs_s.rearrange("p (q t) -> p q t", q=4),
                cs[:, :, :].rearrange("q p t -> p q t"),
            )
            nc.gpsimd.dma_start(
                sn_s.rearrange("p (q t) -> p q t", q=4),
                sn[:, :, :].rearrange("q p t -> p q t"),
            )
            nc.scalar.dma_start(ones_s, ones_f[:, :])
            nc.sync.dma_start(
                wp_s.rearrange("p h (n t) -> p h n t", n=4),
                wp.rearrange("h n p t -> p h n t"),
            )

            # per-batch activation tiles (bufs=2 cycles across batches)
            QKH = [None] * B     # [128, 2(q|k), HPC, T]
            VH = [None] * B
            YT = [None] * B

            def stage_a(u):
                """bf16 qkv + rope for query block u (512 tokens)."""
                b, j = divmod(u, NTB_B)
                if j == 0:
                    QKH[b] = qkp.tile([128, 2, HPC, T], BF16, tag="QKH", name="QKH")
                    VH[b] = vpool.tile([128, NKT, HPC * D], BF16, tag="VH", name="VH")
                    YT[b] = ypool.tile([128, HPC, T], BF16, tag="yT", name="yT")
                if u + 1 < NU:
                    xbt[u + 1] = xp.tile([128, NKT, TBS], BF16, tag="xb", name="xbt")
                    # split across two queues: 2MB on one ~50GB/s queue is
                    # marginal against the ~41us per-block cadence
                    nc.gpsimd.dma_start(xbt[u + 1][:, 0:8, :], xbd[u + 1][:, 0:8, :])
                    nc.sync.dma_start(xbt[u + 1][:, 8:16, :], xbd[u + 1][:, 8:16, :])
                xs = xbt[u]

                # ---- q/k projections, colgroup phases ----
                # colgroups: 0=Qtop 1=Qbot 2=Ktop 3=Kbot ([h0|h1] x 64 dims each)
                tb = rinp.tile([128, 2, TBS], BF16, tag="rin", name="tb")
                bb = rinp.tile([128, 2, TBS], BF16, tag="rin", name="bb")
                if u == 0:
                    # block 0 is DMA-arrival-paced: consume each k-tile for two
                    # colgroups as it lands instead of re-scanning k per cg
                    for half in range(2):
                        psA = pool_a.tile([128, 512], F32, tag="a", name="qkpsA")
                        psB = pool_a.tile([128, 512], F32, tag="a", name="qkpsB")
                        for k in range(NKT):
                            st, sp = (k == 0), (k == NKT - 1)
                            nc.tensor.matmul(
                                psA,
                                wqk_s[:, k, (2 * half) * 128 : (2 * half + 1) * 128],
                                xs[:, k, :], start=st, stop=sp,
                            )
                            nc.tensor.matmul(
                                psB,
                                wqk_s[:, k, (2 * half + 1) * 128 : (2 * half + 2) * 128],
                                xs[:, k, :], start=st, stop=sp,
                            )
                        nc.scalar.activation(
                            tb[:, half, :], psA, mybir.ActivationFunctionType.Copy
                        )
                        nc.scalar.activation(
                            bb[:, half, :], psB, mybir.ActivationFunctionType.Copy
                        )
                else:
                    for cg in range(4):
                        ps = pool_a.tile([128, 512], F32, tag="a", name="qkps")
                        for k in range(NKT):
                            nc.tensor.matmul(
                                ps,
                                wqk_s[:, k, cg * 128 : (cg + 1) * 128],
                                xs[:, k, :],
                                start=(k == 0),
                                stop=(k == NKT - 1),
                            )
                        dst = (tb, bb)[cg % 2]
                        nc.scalar.activation(
                            dst[:, cg // 2, :], ps, mybir.ActivationFunctionType.Copy
                        )

                # ---- rope + regather ----
                tcols = slice(j * TBS, (j + 1) * TBS)
                c_b = cs_s[:, tcols][:, None, :].broadcast_to([128, 2, TBS])
                s_b = sn_s[:, tcols][:, None, :].broadcast_to([128, 2, TBS])
                t1 = tmpp.tile([128, 2, TBS], BF16, tag="t")
                nc.vector.tensor_mul(t1, tb, c_b)
                t2 = tmpp.tile([128, 2, TBS], BF16, tag="t")
                nc.vector.tensor_mul(t2, bb, s_b)
                t3 = tmpp.tile([128, 2, TBS], BF16, tag="t")
                nc.vector.tensor_mul(t3, tb, s_b)
                t4 = tmpp.tile([128, 2, TBS], BF16, tag="t")
                nc.vector.tensor_mul(t4, bb, c_b)
                rtop = rtp.tile([128, 2, TBS], BF16, tag="rt")
                nc.vector.tensor_sub(rtop, t1, t2)
                rbot = rtp.tile([128, 2, TBS], BF16, tag="rt")
                nc.vector.tensor_add(rbot, t3, t4)
                # head-0 tops and head-1 bots stay on their partitions (DVE);
                # the other two quarters cross partitions (DMA on sync).
                nc.vector.tensor_copy(QKH[b][0:64, :, 0, tcols], rtop[0:64, :, :])
                nc.vector.tensor_copy(QKH[b][64:128, :, 1, tcols], rbot[64:128, :, :])
                nc.sync.dma_start(QKH[b][64:128, :, 0, tcols], rbot[0:64, :, :])
                nc.sync.dma_start(QKH[b][0:64, :, 1, tcols], rtop[64:128, :, :])

                # ---- v projection (x-chunk stationary) ----
                for bank in range(2):        # 2 token-chunks of 128 per bank
                    vps = pool_a.tile([128, 512], F32, tag="a", name="vps")
                    for k in range(NKT):
                        for sc in range(2):
                            ch = 2 * bank + sc   # token chunk in block
                            nc.tensor.matmul(
                                vps[:, sc * 256 : (sc + 1) * 256],
                                xs[:, k, ch * 128 : (ch + 1) * 128],
                                wv_s[:, k, :],
                                start=(k == 0 and sc == 0),
                                stop=(k == NKT - 1 and sc == 1),
                            )
                    for sc in range(2):
                        chg = j * 4 + 2 * bank + sc   # chunk in batch
                        nc.scalar.activation(
                            VH[b][:, chg, :],
                            vps[:, sc * 256 : (sc + 1) * 256],
                            mybir.ActivationFunctionType.Copy,
                        )

            def stage_b(u, hsel):
                """causal attention for query block u, one head.

                The denominator is accumulated OFF the PE: gpsimd adds P
                chunks pairwise into f32 tiles; one f32 ones-matmul per head
                broadcasts the cross-partition sum.  pv matmuls lag two
                S-pairs behind (pend), carrying across the head boundary.
                """
                b, j = divmod(u, NTB_B)
                n_k = 4 * (j + 1)
                pend = []
                for h in hsel:
                    stripes = []
                    for _si in range((n_k + 7) // 8):
                        p_stripe = ppool.tile([128, 4096], BF16, tag="P")
                        stripes.append(p_stripe)

                    def poff(m):
                        return 128 * max(0, m - 4 * j)

                    def pchunk(m, stripes=stripes):
                        o = poff(m)
                        return stripes[m // 8][:, (m % 8) * 512 + o : (m % 8) * 512 + 512]

                    pv = pool_a.tile([128, 512], F32, tag="a", name="pv")
                    den = pool_dv.tile([128, 512], F32, tag="dv", name="den")

                    def denpv_pair(g, h=h, pv=pv, den=den, pchunk=pchunk):
                        for w in (0, 1):
                            m = 2 * g + w
                            o = poff(m)
                            nc.tensor.matmul(
                                den[:, o:512], ones_s, pchunk(m),
                                start=(m == 0), stop=(m == n_k - 1),
                            )
                            nc.tensor.matmul(
                                pv[:, o:512], VH[b][:, m, h * D : (h + 1) * D], pchunk(m),
                                start=(m == 0), stop=(m == n_k - 1),
                            )
                        if 2 * g + 1 == n_k - 1:
                            rc = rcp.tile([128, 512], F32, tag="rc", name="rc")
                            nc.vector.reciprocal_approx_fast(out=rc, in_=den)
                            qsl = slice(j * TBS, (j + 1) * TBS)
                            nc.vector.tensor_mul(YT[b][:, h, qsl], pv, rc)

                    for m in range(n_k):
                        o = poff(m)
                        sg = pool_sg.tile([128, 512], F32, tag="sg", name="sg")
                        nc.tensor.matmul(
                            sg[:, o:512],
                            QKH[b][:, 1, h, m * 128 : (m + 1) * 128],
                            QKH[b][:, 0, h, j * TBS + o : (j + 1) * TBS],
                            start=True,
                            stop=True,
                        )
                        sbase = (m % 8) * 512
                        stripe = stripes[m // 8]
                        nc.scalar.activation(
                            stripe[:, sbase + o : sbase + 512], sg[:, o:512],
                            mybir.ActivationFunctionType.Exp, scale=INV_SQRT_D,
                        )
                        r = m - 4 * j
                        if r >= 0:
                            ck = stripe[
                                :, sbase + 128 * r : sbase + 128 * r + 128
                            ]
                            nc.gpsimd.affine_select(
                                out=ck,
                                in_=ck,
                                compare_op=mybir.AluOpType.is_ge,
                                fill=0.0,
                                base=0,
                                pattern=[[1, 128]],
                                channel_multiplier=-1,
                            )
                        if m % 2 == 1:
                            pend.append((denpv_pair, m // 2))
                            if len(pend) > 3:
                                fn, ga = pend.pop(0)
                                fn(ga)
                return pend

            def stage_c(u, isel):
                """partial c_proj for query block u (bf16 output).

                For the two drain-phase blocks (no stage_a running) the PSUM
                evictions alternate Scalar/Vector to halve the tail latency.
                """
                b, j = divmod(u, NTB_B)
                alt = u >= NU - 2
                for i4 in isel:
                    i = j * 4 + i4
                    row0 = b * T + i * 128
                    ot = outp.tile([128, C], BF16, tag="o")
                    for n2 in range(C // 512):
                        ps = pool_a.tile([128, 512], F32, tag="a", name="cps")
                        for hh in range(HPC):
                            nc.tensor.matmul(
                                ps,
                                YT[b][:, hh, i * 128 : (i + 1) * 128],
                                wp_s[:, hh, n2 * 512 : (n2 + 1) * 512],
                                start=(hh == 0),
                                stop=(hh == HPC - 1),
                            )
                        if alt and n2 % 2 == 0:
                            nc.scalar.activation(
                                ot[:, n2 * 512 : (n2 + 1) * 512], ps,
                                mybir.ActivationFunctionType.Copy,
                            )
                        else:
                            nc.vector.tensor_copy(ot[:, n2 * 512 : (n2 + 1) * 512], ps)
                    nc.sync.dma_start(out_d[row0 : row0 + 128, :], ot)

            def flush(pend):
                while pend:
                    fn, ga = pend.pop(0)
                    fn(ga)

            for step in range(NU + 2):
                if step < NU:
                    stage_a(step)
                if 1 <= step <= NU:
                    flush(stage_b(step - 1, (0,)))
                if step >= 2:
                    stage_c(step - 2, (0, 1))
                if 1 <= step <= NU:
                    flush(stage_b(step - 1, (1,)))
                if step >= 2:
                    stage_c(step - 2, (2, 3))

    nc.compile()
    return nc


def _host_prep(x, w_atten, w_proj):
    """Build the shared + per-core input arrays."""
    x = np.asarray(x, dtype=np.float32)
    w_atten = np.asarray(w_atten, dtype=np.float32)
    w_proj = np.asarray(w_proj, dtype=np.float32)

    # xb[u, p, k, t] = x[token u*512+t, channel k*128+p]
    xf = x.reshape(B * NTB_B, TBS, NKT, 128)
    xb = np.ascontiguousarray(xf.transpose(0, 3, 2, 1)).astype(NP_BF16)

    wq = w_atten[0:C]
    wk = w_atten[C : 2 * C]
    wv_full = w_atten[2 * C : 3 * C]

    # rope tables: theta_i = base^(-2i/D)
    theta = 1.0 / (10000.0 ** (np.arange(0, D, 2, dtype=np.float64) / D))  # [64]
    tpos = np.arange(T, dtype=np.float64)
    ang = np.outer(theta, tpos)  # [64, T]
    cs_half = np.cos(ang).astype(np.float32)
    sn_half = np.sin(ang).astype(np.float32)
    cs = np.concatenate([cs_half, cs_half], axis=0).astype(NP_BF16)  # [128, T]
    sn = np.concatenate([sn_half, sn_half], axis=0).astype(NP_BF16)
    cs = np.ascontiguousarray(cs.reshape(128, 4, 512).transpose(1, 0, 2))
    sn = np.ascontiguousarray(sn.reshape(128, 4, 512).transpose(1, 0, 2))

    ones_f = np.ones((128, 128), dtype=np.float32).astype(NP_BF16)

    top_idx = np.arange(0, D, 2)   # 64
    bot_idx = np.arange(1, D, 2)

    in_maps = []
    for c in range(NC_CORES):
        heads = [HPC * c + h for h in range(HPC)]
        # colgroups: 0 (tops of q), 1 (bots of q), 2/3 same for k
        fb = []
        for wmat in (wq, wk):
            for idx in (top_idx, bot_idx):
                rows = np.concatenate([wmat[hh * D + idx] for hh in heads], axis=0)
                fb.append(rows)  # [128, C]
        w_qk_c = np.concatenate(fb, axis=0)  # [512, C]
        wqk_dev = np.ascontiguousarray(
            w_qk_c.T.reshape(NKT, 128, 4 * 128).astype(NP_BF16)
        )
        w_v_c = np.concatenate([wv_full[hh * D : (hh + 1) * D] for hh in heads], axis=0)
        wv_dev = np.ascontiguousarray(
            w_v_c.T.reshape(NKT, 128, HPC * D).astype(NP_BF16)
        )
        cols = np.concatenate([np.arange(hh * D, (hh + 1) * D) for hh in heads])
        w_p_c = np.ascontiguousarray(w_proj[:, cols].T)  # [256, C]
        wp_dev = np.ascontiguousarray(
            w_p_c.reshape(HPC, 128, C // 512, 512).transpose(0, 2, 1, 3).astype(NP_BF16)
        )
        in_maps.append(
            {
                "xb": xb,
                "wqk": wqk_dev,
                "wv": wv_dev,
                "wp": wp_dev,
                "cs": cs,
                "sn": sn,
                "ones_f": ones_f,
            }
        )
    return in_maps


def _execute(in_maps, trace=False, trace_kwargs=None):
    if "nc" not in _CACHE:
        _CACHE["nc"] = _build_program()
    nc = _CACHE["nc"]
    kwargs = {}
    if trace:
        _install_ntff_hook()
        kwargs["trace"] = True
        if trace_kwargs:
            kwargs.update(trace_kwargs)
    return run_bass_kernel_spmd(nc, in_maps, core_ids=list(range(NC_CORES)), **kwargs)


def _install_ntff_hook():
    """Restore the axon NTFF profile hook (the container's antenv lacks it)."""
    import types

    if "antenv.axon_hooks" in sys.modules:
        return
    mod = types.ModuleType("antenv.axon_hooks")
    mod._hook = None

    def set_axon_ntff_profile_hook(h):
        mod._hook = h

    def get_axon_ntff_profile_hook():
        if mod._hook is None:
            try:
                from trn_agent_boot.trn_boot import _ntff_profile_via_ctypes

                mod._hook = _ntff_profile_via_ctypes("/opt/axon/libaxon_pjrt.so")
            except Exception:
                mod._hook = None
        return mod._hook

    mod.set_axon_ntff_profile_hook = set_axon_ntff_profile_hook
    mod.get_axon_ntff_profile_hook = get_axon_ntff_profile_hook
    sys.modules["antenv.axon_hooks"] = mod


def kernel(x, w_atten, w_proj):
    in_maps = _host_prep(x, w_atten, w_proj)
    res = _execute(in_maps)
    total = res.results[0]["out"].astype(np.float32)
    for c in range(1, NC_CORES):
        total = total + res.results[c]["out"].astype(np.float32)
    return total.reshape(B, T, C)


# revision 24
# speedup vs baseline: 1.0337x; 1.0337x over previous
"""Trainium2 Bass kernel for nn_AttentionHead (B=2, T=2048, C=2048, H=16 heads, D=128).

Sharding: tensor-parallel over heads - 2 heads per NeuronCore (8 cores).
Each core computes qkv for its heads, RoPE, causal softmax attention, and a
partial c_proj contribution; the host sums the 8 partial outputs.

v8 vs v2-baseline (414us -> ~350us measured):
  - All matmuls bf16.  (fp8/DoubleRow was tried and reverted: softmax
    averaging shrinks signal and quantization noise at the same rate, so
    single-pass fp8 anywhere in the qkv path costs ~2-3e-2 relmax -- over
    the 2e-2 gate.  Full hi/lo compensation needs 3 fp8 passes = 1.5x bf16
    time.  gpsimd den-accumulation was also tried and reverted: gpsimd
    tensor ops run at ~55 G elem/s, 1.2us per [128,512] add.)
  - Few, large DMAs: the ~0.6us-per-dma_start sequencer cost was starving
    the Scalar (70us) and Sync (120us) engines in v2.  Startup transfers
    are spread across the three DMA-capable queues (sync/scalar/gpsimd,
    ~50GB/s each) in consumption order; x blocks prefetched one block
    ahead on two queues.
  - stage_a in colgroup phases (Qtop/Qbot/Ktop/Kbot) with [128,512] moving
    operands: fewer, larger matmuls and fewer evictions than the v2
    half-block layout.
  - sg pool 3x[128,512] with per-chunk exp (finer PSUM release for the
    S -> exp chain), pool_a 3 bufs; c_proj evictions on DVE except the
    drain-phase blocks which alternate Scalar/Vector; partial outputs
    written bf16 (halves out-DMA bytes; host sums in f32).
"""

import sys

sys.path.insert(0, "/opt/trn_rl_repo")

import ml_dtypes
import numpy as np

import concourse.mybir as mybir
import concourse.tile as tile
from concourse import bacc
from concourse.bass_utils import run_bass_kernel_spmd

F32 = mybir.dt.float32
BF16 = mybir.dt.bfloat16
NP_BF16 = np.dtype(ml_dtypes.bfloat16)

B, T, C, H, D = 2, 2048, 2048, 16, 128
NC_CORES = 8
HPC = H // NC_CORES            # heads per core = 2
BT = B * T                     # 4096
NKT = C // 128                 # 16 contraction tiles (128-chan)
TBS = 512                      # query block size
NTB_B = T // TBS               # 4 query blocks per batch
INV_SQRT_D = 1.0 / float(np.sqrt(D))

_CACHE = {}


def _build_program():
    nc = bacc.Bacc(None)

    # xb: per block, [128 chan-lo, ktile, 512 tok] bf16
    xbd = nc.dram_tensor("xb", [B * NTB_B, 128, NKT, TBS], BF16, kind="ExternalInput")
    wqk = nc.dram_tensor("wqk", [NKT, 128, 4 * 128], BF16, kind="ExternalInput")
    wv = nc.dram_tensor("wv", [NKT, 128, HPC * D], BF16, kind="ExternalInput")
    wp = nc.dram_tensor("wp", [HPC, C // 512, 128, 512], BF16, kind="ExternalInput")
    cs = nc.dram_tensor("cs", [4, 128, 512], BF16, kind="ExternalInput")
    sn = nc.dram_tensor("sn", [4, 128, 512], BF16, kind="ExternalInput")
    ones_f = nc.dram_tensor("ones_f", [128, 128], BF16, kind="ExternalInput")
    out_d = nc.dram_tensor("out", [BT, C], BF16, kind="ExternalOutput")

    NU = B * NTB_B              # 8 query blocks across both batches

    with tile.TileContext(nc) as tc:
        with (
            tc.tile_pool(name="const", bufs=1) as constp,
            tc.tile_pool(name="xp", bufs=2) as xp,         # x block tiles
            tc.tile_pool(name="qk", bufs=2) as qkp,        # QH/KH per batch
            tc.tile_pool(name="vp", bufs=2) as vpool,      # VH per batch
            tc.tile_pool(name="yp", bufs=2) as ypool,      # yT per batch
            tc.tile_pool(name="rin", bufs=3) as rinp,      # rope inputs (tops/bots)
            tc.tile_pool(name="tmp", bufs=4) as tmpp,      # rope temporaries
            tc.tile_pool(name="rt", bufs=3) as rtp,        # rope outputs
            tc.tile_pool(name="pp", bufs=3) as ppool,      # P stripes
            tc.tile_pool(name="rc", bufs=2) as rcp,        # reciprocal out
            tc.tile_pool(name="outp", bufs=2) as outp,     # c_proj staging
            tc.tile_pool(name="ps_a", bufs=3, space="PSUM") as pool_a,
            tc.tile_pool(name="ps_sg", bufs=3, space="PSUM") as pool_sg,
            tc.tile_pool(name="ps_dv", bufs=2, space="PSUM") as pool_dv,
        ):
            # ---- constants ----
            # Startup is DMA-latency-bound: ~3.5MB (wqk + x-block-0 + wv +
            # rope tables) must land before stage_a(0) can finish, and a
            # single DMA queue moves only ~50GB/s.  Spread the startup
            # transfers across the three DMA-capable queues (sync, scalar,
            # gpsimd), interleaved in k-tile consumption order.
            wqk_s = constp.tile([128, NKT, 4 * 128], BF16, tag="wqk")
            wv_s = constp.tile([128, NKT, HPC * D], BF16, tag="wv")
            cs_s = constp.tile([128, T], BF16, tag="cs")
            sn_s = constp.tile([128, T], BF16, tag="sn")
            ones_s = constp.tile([128, 128], BF16, tag="ones")
            wp_s = constp.tile([128, HPC, C], BF16, tag="wp")
            xbt = [None] * NU
            xbt[0] = xp.tile([128, NKT, TBS], BF16, tag="xb", name="xbt0")

            def wqk_chunk(eng, ks):
                eng.dma_start(wqk_s[:, ks, :], wqk[ks].rearrange("k p m -> p k m"))

            def x0_chunk(eng, ks):
                eng.dma_start(xbt[0][:, ks, :], xbd[0][:, ks, :])

            engs = (nc.sync, nc.scalar, nc.gpsimd)
            for c6 in range(3):
                wqk_chunk(engs[c6], slice(2 * c6, 2 * c6 + 2))
                x0_chunk(engs[c6], slice(2 * c6, 2 * c6 + 2))
            for c6 in range(3):
                wqk_chunk(engs[c6], slice(6 + 2 * c6, 8 + 2 * c6))
                x0_chunk(engs[c6], slice(6 + 2 * c6, 8 + 2 * c6))
            wqk_chunk(nc.sync, slice(12, 14))
            wqk_chunk(nc.scalar, slice(14, 16))
            x0_chunk(nc.sync, slice(12, 14))
            x0_chunk(nc.scalar, slice(14, 16))
            nc.sync.dma_start(wv_s, wv.rearrange("k p t -> p k t"))
            nc.scalar.dma_start(
                cs_s.rearrange("p (q t) -> p q t", q=4),
                cs[:, :, :].rearrange("q p t -> p q t"),
            )
            nc.gpsimd.dma_start(
                sn_s.rearrange("p (q t) -> p q t", q=4),
                sn[:, :, :].rearrange("q p t -> p q t"),
            )
            nc.scalar.dma_start(ones_s, ones_f[:, :])
            nc.sync.dma_start(
                wp_s.rearrange("p h (n t) -> p h n t", n=4),
                wp.rearrange("h n p t -> p h n t"),
            )

            # per-batch activation tiles (bufs=2 cycles across batches)
            QKH = [None] * B     # [128, 2(q|k), HPC, T]
            VH = [None] * B
            YT = [None] * B

            def stage_a(u):
                """bf16 qkv + rope for query block u (512 tokens)."""
                b, j = divmod(u, NTB_B)
                if j == 0:
                    QKH[b] = qkp.tile([128, 2, HPC, T], BF16, tag="QKH", name="QKH")
                    VH[b] = vpool.tile([128, NKT, HPC * D], BF16, tag="VH", name="VH")
                    YT[b] = ypool.tile([128, HPC, T], BF16, tag="yT", name="yT")
                if u + 1 < NU:
                    xbt[u + 1] = xp.tile([128, NKT, TBS], BF16, tag="xb", name="xbt")
                    # split across two queues: 2MB on one ~50GB/s queue is
                    # marginal against the ~41us per-block cadence
                    nc.gpsimd.dma_start(xbt[u + 1][:, 0:8, :], xbd[u + 1][:, 0:8, :])
                    nc.sync.dma_start(xbt[u + 1][:, 8:16, :], xbd[u + 1][:, 8:16, :])
                xs = xbt[u]

                # ---- q/k projections, colgroup phases ----
                # colgroups: 0=Qtop 1=Qbot 2=Ktop 3=Kbot ([h0|h1] x 64 dims each)
                tb = rinp.tile([128, 2, TBS], BF16, tag="rin", name="tb")
                bb = rinp.tile([128, 2, TBS], BF16, tag="rin", name="bb")
                for cg in range(4):
                    ps = pool_a.tile([128, 512], F32, tag="a", name="qkps")
                    for k in range(NKT):
                        nc.tensor.matmul(
                            ps,
                            wqk_s[:, k, cg * 128 : (cg + 1) * 128],
                            xs[:, k, :],
                            start=(k == 0),
                            stop=(k == NKT - 1),
                        )
                    dst = (tb, bb)[cg % 2]
                    nc.scalar.activation(
                        dst[:, cg // 2, :], ps, mybir.ActivationFunctionType.Copy
                    )

                # ---- rope + regather ----
                tcols = slice(j * TBS, (j + 1) * TBS)
                c_b = cs_s[:, tcols][:, None, :].broadcast_to([128, 2, TBS])
                s_b = sn_s[:, tcols][:, None, :].broadcast_to([128, 2, TBS])
                t1 = tmpp.tile([128, 2, TBS], BF16, tag="t")
                nc.vector.tensor_mul(t1, tb, c_b)
                t2 = tmpp.tile([128, 2, TBS], BF16, tag="t")
                nc.vector.tensor_mul(t2, bb, s_b)
                t3 = tmpp.tile([128, 2, TBS], BF16, tag="t")
                nc.vector.tensor_mul(t3, tb, s_b)
                t4 = tmpp.tile([128, 2, TBS], BF16, tag="t")
                nc.vector.tensor_mul(t4, bb, c_b)
                rtop = rtp.tile([128, 2, TBS], BF16, tag="rt")
                nc.vector.tensor_sub(rtop, t1, t2)
                rbot = rtp.tile([128, 2, TBS], BF16, tag="rt")
                nc.vector.tensor_add(rbot, t3, t4)
                # head-0 tops and head-1 bots stay on their partitions (DVE);
                # the other two quarters cross partitions (DMA on sync).
                nc.vector.tensor_copy(QKH[b][0:64, :, 0, tcols], rtop[0:64, :, :])
                nc.vector.tensor_copy(QKH[b][64:128, :, 1, tcols], rbot[64:128, :, :])
                nc.sync.dma_start(QKH[b][64:128, :, 0, tcols], rbot[0:64, :, :])
                nc.sync.dma_start(QKH[b][0:64, :, 1, tcols], rtop[64:128, :, :])

                # ---- v projection (x-chunk stationary) ----
                for bank in range(2):        # 2 token-chunks of 128 per bank
                    vps = pool_a.tile([128, 512], F32, tag="a", name="vps")
                    for k in range(NKT):
                        for sc in range(2):
                            ch = 2 * bank + sc   # token chunk in block
                            nc.tensor.matmul(
                                vps[:, sc * 256 : (sc + 1) * 256],
                                xs[:, k, ch * 128 : (ch + 1) * 128],
                                wv_s[:, k, :],
                                start=(k == 0 and sc == 0),
                                stop=(k == NKT - 1 and sc == 1),
                            )
                    for sc in range(2):
                        chg = j * 4 + 2 * bank + sc   # chunk in batch
                        nc.scalar.activation(
                            VH[b][:, chg, :],
                            vps[:, sc * 256 : (sc + 1) * 256],
                            mybir.ActivationFunctionType.Copy,
                        )

            def stage_b(u, hsel):
                """causal attention for query block u, one head.

                The denominator is accumulated OFF the PE: gpsimd adds P
                chunks pairwise into f32 tiles; one f32 ones-matmul per head
                broadcasts the cross-partition sum.  pv matmuls lag two
                S-pairs behind (pend), carrying across the head boundary.
                """
                b, j = divmod(u, NTB_B)
                n_k = 4 * (j + 1)
                pend = []
                for h in hsel:
                    stripes = []
                    for _si in range((n_k + 7) // 8):
                        p_stripe = ppool.tile([128, 4096], BF16, tag="P")
                        stripes.append(p_stripe)

                    def poff(m):
                        return 128 * max(0, m - 4 * j)

                    def pchunk(m, stripes=stripes):
                        o = poff(m)
                        return stripes[m // 8][:, (m % 8) * 512 + o : (m % 8) * 512 + 512]

                    pv = pool_a.tile([128, 512], F32, tag="a", name="pv")
                    den = pool_dv.tile([128, 512], F32, tag="dv", name="den")

                    def denpv_pair(g, h=h, pv=pv, den=den, pchunk=pchunk):
                        for w in (0, 1):
                            m = 2 * g + w
                            o = poff(m)
                            nc.tensor.matmul(
                                den[:, o:512], ones_s, pchunk(m),
                                start=(m == 0), stop=(m == n_k - 1),
                            )
                            nc.tensor.matmul(
                                pv[:, o:512], VH[b][:, m, h * D : (h + 1) * D], pchunk(m),
                                start=(m == 0), stop=(m == n_k - 1),
                            )
                        if 2 * g + 1 == n_k - 1:
                            rc = rcp.tile([128, 512], F32, tag="rc", name="rc")
                            nc.vector.reciprocal_approx_fast(out=rc, in_=den)
                            qsl = slice(j * TBS, (j + 1) * TBS)
                            nc.vector.tensor_mul(YT[b][:, h, qsl], pv, rc)

                    for m in range(n_k):
                        o = poff(m)
                        sg = pool_sg.tile([128, 512], F32, tag="sg", name="sg")
                        nc.tensor.matmul(
                            sg[:, o:512],
                            QKH[b][:, 1, h, m * 128 : (m + 1) * 128],
                            QKH[b][:, 0, h, j * TBS + o : (j + 1) * TBS],
                            start=True,
                            stop=True,
                        )
                        sbase = (m % 8) * 512
                        stripe = stripes[m // 8]
                        nc.scalar.activation(
                            stripe[:, sbase + o : sbase + 512], sg[:, o:512],
                            mybir.ActivationFunctionType.Exp, scale=INV_SQRT_D,
                        )
                        r = m - 4 * j
                        if r >= 0:
                            ck = stripe[
                                :, sbase + 128 * r : sbase + 128 * r + 128
                            ]
                            nc.gpsimd.affine_select(
                                out=ck,
                                in_=ck,
                                compare_op=mybir.AluOpType.is_ge,
                                fill=0.0,
                                base=0,
                                pattern=[[1, 128]],
                                channel_multiplier=-1,
                            )
                        if m % 2 == 1:
                            pend.append((denpv_pair, m // 2))
                            if len(pend) > 3:
                                fn, ga = pend.pop(0)
                                fn(ga)
                return pend

            def stage_c(u, isel):
                """partial c_proj for query block u (bf16 output).

                For the two drain-phase blocks (no stage_a running) the PSUM
                evictions alternate Scalar/Vector to halve the tail latency.
                """
                b, j = divmod(u, NTB_B)
                alt = u >= NU - 2
                for i4 in isel:
                    i = j * 4 + i4
                    row0 = b * T + i * 128
                    ot = outp.tile([128, C], BF16, tag="o")
                    for n2 in range(C // 512):
                        ps = pool_a.tile([128, 512], F32, tag="a", name="cps")
                        for hh in range(HPC):
                            nc.tensor.matmul(
                                ps,
                                YT[b][:, hh, i * 128 : (i + 1) * 128],
                                wp_s[:, hh, n2 * 512 : (n2 + 1) * 512],
                                start=(hh == 0),
                                stop=(hh == HPC - 1),
                            )
                        if alt and n2 % 2 == 0:
                            nc.scalar.activation(
                                ot[:, n2 * 512 : (n2 + 1) * 512], ps,
                                mybir.ActivationFunctionType.Copy,
                            )
                        else:
                            nc.vector.tensor_copy(ot[:, n2 * 512 : (n2 + 1) * 512], ps)
                    nc.sync.dma_start(out_d[row0 : row0 + 128, :], ot)

            def flush(pend):
                while pend:
                    fn, ga = pend.pop(0)
                    fn(ga)

            for step in range(NU + 2):
                if step < NU:
                    stage_a(step)
                if 1 <= step <= NU:
                    flush(stage_b(step - 1, (0,)))
                if step >= 2:
                    stage_c(step - 2, (0, 1))
                if 1 <= step <= NU:
                    flush(stage_b(step - 1, (1,)))
                if step >= 2:
                    stage_c(step - 2, (2, 3))

    nc.compile()
    return nc


def _host_prep(x, w_atten, w_proj):
    """Build the shared + per-core input arrays."""
    x = np.asarray(x, dtype=np.float32)
    w_atten = np.asarray(w_atten, dtype=np.float32)
    w_proj = np.asarray(w_proj, dtype=np.float32)

    # xb[u, p, k, t] = x[token u*512+t, channel k*128+p]
    xf = x.reshape(B * NTB_B, TBS, NKT, 128)
    xb = np.ascontiguousarray(xf.transpose(0, 3, 2, 1)).astype(NP_BF16)

    wq = w_atten[0:C]
    wk = w_atten[C : 2 * C]
    wv_full = w_atten[2 * C : 3 * C]

    # rope tables: theta_i = base^(-2i/D)
    theta = 1.0 / (10000.0 ** (np.arange(0, D, 2, dtype=np.float64) / D))  # [64]
    tpos = np.arange(T, dtype=np.float64)
    ang = np.outer(theta, tpos)  # [64, T]
    cs_half = np.cos(ang).astype(np.float32)
    sn_half = np.sin(ang).astype(np.float32)
    cs = np.concatenate([cs_half, cs_half], axis=0).astype(NP_BF16)  # [128, T]
    sn = np.concatenate([sn_half, sn_half], axis=0).astype(NP_BF16)
    cs = np.ascontiguousarray(cs.reshape(128, 4, 512).transpose(1, 0, 2))
    sn = np.ascontiguousarray(sn.reshape(128, 4, 512).transpose(1, 0, 2))

    ones_f = np.ones((128, 128), dtype=np.float32).astype(NP_BF16)

    top_idx = np.arange(0, D, 2)   # 64
    bot_idx = np.arange(1, D, 2)

    in_maps = []
    for c in range(NC_CORES):
        heads = [HPC * c + h for h in range(HPC)]
        # colgroups: 0 (tops of q), 1 (bots of q), 2/3 same for k
        fb = []
        for wmat in (wq, wk):
            for idx in (top_idx, bot_idx):
                rows = np.concatenate([wmat[hh * D + idx] for hh in heads], axis=0)
                fb.append(rows)  # [128, C]
        w_qk_c = np.concatenate(fb, axis=0)  # [512, C]
        wqk_dev = np.ascontiguousarray(
            w_qk_c.T.reshape(NKT, 128, 4 * 128).astype(NP_BF16)
        )
        w_v_c = np.concatenate([wv_full[hh * D : (hh + 1) * D] for hh in heads], axis=0)
        wv_dev = np.ascontiguousarray(
            w_v_c.T.reshape(NKT, 128, HPC * D).astype(NP_BF16)
        )
        cols = np.concatenate([np.arange(hh * D, (hh + 1) * D) for hh in heads])
        w_p_c = np.ascontiguousarray(w_proj[:, cols].T)  # [256, C]
        wp_dev = np.ascontiguousarray(
            w_p_c.reshape(HPC, 128, C // 512, 512).transpose(0, 2, 1, 3).astype(NP_BF16)
        )
        in_maps.append(
            {
                "xb": xb,
                "wqk": wqk_dev,
                "wv": wv_dev,
                "wp": wp_dev,
                "cs": cs,
                "sn": sn,
                "ones_f": ones_f,
            }
        )
    return in_maps


def _execute(in_maps, trace=False, trace_kwargs=None):
    if "nc" not in _CACHE:
        _CACHE["nc"] = _build_program()
    nc = _CACHE["nc"]
    kwargs = {}
    if trace:
        _install_ntff_hook()
        kwargs["trace"] = True
        if trace_kwargs:
            kwargs.update(trace_kwargs)
    return run_bass_kernel_spmd(nc, in_maps, core_ids=list(range(NC_CORES)), **kwargs)


def _install_ntff_hook():
    """Restore the axon NTFF profile hook (the container's antenv lacks it)."""
    import types

    if "antenv.axon_hooks" in sys.modules:
        return
    mod = types.ModuleType("antenv.axon_hooks")
    mod._hook = None

    def set_axon_ntff_profile_hook(h):
        mod._hook = h

    def get_axon_ntff_profile_hook():
        if mod._hook is None:
            try:
                from trn_agent_boot.trn_boot import _ntff_profile_via_ctypes

                mod._hook = _ntff_profile_via_ctypes("/opt/axon/libaxon_pjrt.so")
            except Exception:
                mod._hook = None
        return mod._hook

    mod.set_axon_ntff_profile_hook = set_axon_ntff_profile_hook
    mod.get_axon_ntff_profile_hook = get_axon_ntff_profile_hook
    sys.modules["antenv.axon_hooks"] = mod


def kernel(x, w_atten, w_proj):
    in_maps = _host_prep(x, w_atten, w_proj)
    res = _execute(in_maps)
    total = res.results[0]["out"].astype(np.float32)
    for c in range(1, NC_CORES):
        total = total + res.results[c]["out"].astype(np.float32)
    return total.reshape(B, T, C)


# revision 25
# speedup vs baseline: 1.0387x; 1.0049x over previous
"""Trainium2 Bass kernel for nn_AttentionHead (B=2, T=2048, C=2048, H=16 heads, D=128).

Sharding: tensor-parallel over heads - 2 heads per NeuronCore (8 cores).
Each core computes qkv for its heads, RoPE, causal softmax attention, and a
partial c_proj contribution; the host sums the 8 partial outputs.

v8 vs v2-baseline (414us -> ~350us measured):
  - All matmuls bf16.  (fp8/DoubleRow was tried and reverted: softmax
    averaging shrinks signal and quantization noise at the same rate, so
    single-pass fp8 anywhere in the qkv path costs ~2-3e-2 relmax -- over
    the 2e-2 gate.  Full hi/lo compensation needs 3 fp8 passes = 1.5x bf16
    time.  gpsimd den-accumulation was also tried and reverted: gpsimd
    tensor ops run at ~55 G elem/s, 1.2us per [128,512] add.)
  - Few, large DMAs: the ~0.6us-per-dma_start sequencer cost was starving
    the Scalar (70us) and Sync (120us) engines in v2.  Startup transfers
    are spread across the three DMA-capable queues (sync/scalar/gpsimd,
    ~50GB/s each) in consumption order; x blocks prefetched one block
    ahead on two queues.
  - stage_a in colgroup phases (Qtop/Qbot/Ktop/Kbot) with [128,512] moving
    operands: fewer, larger matmuls and fewer evictions than the v2
    half-block layout.
  - sg pool 3x[128,512] with per-chunk exp (finer PSUM release for the
    S -> exp chain), pool_a 3 bufs; c_proj evictions on DVE except the
    drain-phase blocks which alternate Scalar/Vector; partial outputs
    written bf16 (halves out-DMA bytes; host sums in f32).
"""

import sys

sys.path.insert(0, "/opt/trn_rl_repo")

import ml_dtypes
import numpy as np

import concourse.mybir as mybir
import concourse.tile as tile
from concourse import bacc
from concourse.bass_utils import run_bass_kernel_spmd

F32 = mybir.dt.float32
BF16 = mybir.dt.bfloat16
NP_BF16 = np.dtype(ml_dtypes.bfloat16)

B, T, C, H, D = 2, 2048, 2048, 16, 128
NC_CORES = 8
HPC = H // NC_CORES            # heads per core = 2
BT = B * T                     # 4096
NKT = C // 128                 # 16 contraction tiles (128-chan)
TBS = 512                      # query block size
NTB_B = T // TBS               # 4 query blocks per batch
INV_SQRT_D = 1.0 / float(np.sqrt(D))

_CACHE = {}


def _build_program():
    nc = bacc.Bacc(None)

    # xb: per block, [128 chan-lo, ktile, 512 tok] bf16
    xbd = nc.dram_tensor("xb", [B * NTB_B, 128, NKT, TBS], BF16, kind="ExternalInput")
    wqk = nc.dram_tensor("wqk", [NKT, 128, 4 * 128], BF16, kind="ExternalInput")
    wv = nc.dram_tensor("wv", [NKT, 128, HPC * D], BF16, kind="ExternalInput")
    wp = nc.dram_tensor("wp", [HPC, C // 512, 128, 512], BF16, kind="ExternalInput")
    cs = nc.dram_tensor("cs", [4, 128, 512], BF16, kind="ExternalInput")
    sn = nc.dram_tensor("sn", [4, 128, 512], BF16, kind="ExternalInput")
    ones_f = nc.dram_tensor("ones_f", [128, 128], BF16, kind="ExternalInput")
    out_d = nc.dram_tensor("out", [BT, C], BF16, kind="ExternalOutput")

    NU = B * NTB_B              # 8 query blocks across both batches

    with tile.TileContext(nc) as tc:
        with (
            tc.tile_pool(name="const", bufs=1) as constp,
            tc.tile_pool(name="xp", bufs=2) as xp,         # x block tiles
            tc.tile_pool(name="qk", bufs=2) as qkp,        # QH/KH per batch
            tc.tile_pool(name="vp", bufs=2) as vpool,      # VH per batch
            tc.tile_pool(name="yp", bufs=2) as ypool,      # yT per batch
            tc.tile_pool(name="rin", bufs=3) as rinp,      # rope inputs (tops/bots)
            tc.tile_pool(name="tmp", bufs=4) as tmpp,      # rope temporaries
            tc.tile_pool(name="rt", bufs=3) as rtp,        # rope outputs
            tc.tile_pool(name="pp", bufs=3) as ppool,      # P stripes
            tc.tile_pool(name="rc", bufs=2) as rcp,        # reciprocal out
            tc.tile_pool(name="outp", bufs=2) as outp,     # c_proj staging
            tc.tile_pool(name="ps_a", bufs=3, space="PSUM") as pool_a,
            tc.tile_pool(name="ps_sg", bufs=3, space="PSUM") as pool_sg,
            tc.tile_pool(name="ps_dv", bufs=2, space="PSUM") as pool_dv,
        ):
            # ---- constants ----
            # Startup is DMA-latency-bound: ~3.5MB (wqk + x-block-0 + wv +
            # rope tables) must land before stage_a(0) can finish, and a
            # single DMA queue moves only ~50GB/s.  Spread the startup
            # transfers across the three DMA-capable queues (sync, scalar,
            # gpsimd), interleaved in k-tile consumption order.
            wqk_s = constp.tile([128, NKT, 4 * 128], BF16, tag="wqk")
            wv_s = constp.tile([128, NKT, HPC * D], BF16, tag="wv")
            cs_s = constp.tile([128, T], BF16, tag="cs")
            sn_s = constp.tile([128, T], BF16, tag="sn")
            ones_s = constp.tile([128, 128], BF16, tag="ones")
            wp_s = constp.tile([128, HPC, C], BF16, tag="wp")
            xbt = [None] * NU
            xbt[0] = xp.tile([128, NKT, TBS], BF16, tag="xb", name="xbt0")

            def wqk_chunk(eng, ks):
                eng.dma_start(wqk_s[:, ks, :], wqk[ks].rearrange("k p m -> p k m"))

            def x0_chunk(eng, ks):
                eng.dma_start(xbt[0][:, ks, :], xbd[0][:, ks, :])

            engs = (nc.sync, nc.scalar, nc.gpsimd)
            for c6 in range(3):
                wqk_chunk(engs[c6], slice(2 * c6, 2 * c6 + 2))
                x0_chunk(engs[c6], slice(2 * c6, 2 * c6 + 2))
            for c6 in range(3):
                wqk_chunk(engs[c6], slice(6 + 2 * c6, 8 + 2 * c6))
                x0_chunk(engs[c6], slice(6 + 2 * c6, 8 + 2 * c6))
            wqk_chunk(nc.sync, slice(12, 14))
            wqk_chunk(nc.scalar, slice(14, 16))
            x0_chunk(nc.sync, slice(12, 14))
            x0_chunk(nc.scalar, slice(14, 16))
            nc.sync.dma_start(wv_s, wv.rearrange("k p t -> p k t"))
            nc.scalar.dma_start(
                cs_s.rearrange("p (q t) -> p q t", q=4),
                cs[:, :, :].rearrange("q p t -> p q t"),
            )
            nc.gpsimd.dma_start(
                sn_s.rearrange("p (q t) -> p q t", q=4),
                sn[:, :, :].rearrange("q p t -> p q t"),
            )
            nc.scalar.dma_start(ones_s, ones_f[:, :])
            nc.sync.dma_start(
                wp_s.rearrange("p h (n t) -> p h n t", n=4),
                wp.rearrange("h n p t -> p h n t"),
            )

            # per-batch activation tiles (bufs=2 cycles across batches)
            QKH = [None] * B     # [128, 2(q|k), HPC, T]
            VH = [None] * B
            YT = [None] * B

            def stage_a(u):
                """bf16 qkv + rope for query block u (512 tokens)."""
                b, j = divmod(u, NTB_B)
                if j == 0:
                    QKH[b] = qkp.tile([128, 2, HPC, T], BF16, tag="QKH", name="QKH")
                    VH[b] = vpool.tile([128, NKT, HPC * D], BF16, tag="VH", name="VH")
                    YT[b] = ypool.tile([128, HPC, T], BF16, tag="yT", name="yT")
                if u + 1 < NU:
                    xbt[u + 1] = xp.tile([128, NKT, TBS], BF16, tag="xb", name="xbt")
                    # split across two queues: 2MB on one ~50GB/s queue is
                    # marginal against the ~41us per-block cadence
                    nc.gpsimd.dma_start(xbt[u + 1][:, 0:8, :], xbd[u + 1][:, 0:8, :])
                    nc.sync.dma_start(xbt[u + 1][:, 8:16, :], xbd[u + 1][:, 8:16, :])
                xs = xbt[u]

                # ---- q/k projections, colgroup phases ----
                # colgroups: 0=Qtop 1=Qbot 2=Ktop 3=Kbot ([h0|h1] x 64 dims each)
                tb = rinp.tile([128, 2, TBS], BF16, tag="rin", name="tb")
                bb = rinp.tile([128, 2, TBS], BF16, tag="rin", name="bb")
                for cg in range(4):
                    ps = pool_a.tile([128, 512], F32, tag="a", name="qkps")
                    for k in range(NKT):
                        nc.tensor.matmul(
                            ps,
                            wqk_s[:, k, cg * 128 : (cg + 1) * 128],
                            xs[:, k, :],
                            start=(k == 0),
                            stop=(k == NKT - 1),
                        )
                    dst = (tb, bb)[cg % 2]
                    nc.scalar.activation(
                        dst[:, cg // 2, :], ps, mybir.ActivationFunctionType.Copy
                    )

                # ---- rope + regather ----
                tcols = slice(j * TBS, (j + 1) * TBS)
                c_b = cs_s[:, tcols][:, None, :].broadcast_to([128, 2, TBS])
                s_b = sn_s[:, tcols][:, None, :].broadcast_to([128, 2, TBS])
                t1 = tmpp.tile([128, 2, TBS], BF16, tag="t")
                nc.vector.tensor_mul(t1, tb, c_b)
                t2 = tmpp.tile([128, 2, TBS], BF16, tag="t")
                nc.vector.tensor_mul(t2, bb, s_b)
                t3 = tmpp.tile([128, 2, TBS], BF16, tag="t")
                nc.vector.tensor_mul(t3, tb, s_b)
                t4 = tmpp.tile([128, 2, TBS], BF16, tag="t")
                nc.vector.tensor_mul(t4, bb, c_b)
                rtop = rtp.tile([128, 2, TBS], BF16, tag="rt")
                nc.vector.tensor_sub(rtop, t1, t2)
                rbot = rtp.tile([128, 2, TBS], BF16, tag="rt")
                nc.vector.tensor_add(rbot, t3, t4)
                # head-0 tops and head-1 bots stay on their partitions (DVE);
                # the other two quarters cross partitions (DMA on sync).
                nc.vector.tensor_copy(QKH[b][0:64, :, 0, tcols], rtop[0:64, :, :])
                nc.vector.tensor_copy(QKH[b][64:128, :, 1, tcols], rbot[64:128, :, :])
                nc.sync.dma_start(QKH[b][64:128, :, 0, tcols], rbot[0:64, :, :])
                nc.sync.dma_start(QKH[b][0:64, :, 1, tcols], rtop[64:128, :, :])

                # ---- v projection (x-chunk stationary) ----
                for bank in range(2):        # 2 token-chunks of 128 per bank
                    vps = pool_a.tile([128, 512], F32, tag="a", name="vps")
                    for k in range(NKT):
                        for sc in range(2):
                            ch = 2 * bank + sc   # token chunk in block
                            nc.tensor.matmul(
                                vps[:, sc * 256 : (sc + 1) * 256],
                                xs[:, k, ch * 128 : (ch + 1) * 128],
                                wv_s[:, k, :],
                                start=(k == 0 and sc == 0),
                                stop=(k == NKT - 1 and sc == 1),
                            )
                    for sc in range(2):
                        chg = j * 4 + 2 * bank + sc   # chunk in batch
                        nc.scalar.activation(
                            VH[b][:, chg, :],
                            vps[:, sc * 256 : (sc + 1) * 256],
                            mybir.ActivationFunctionType.Copy,
                        )

            def stage_b(u, hsel):
                """causal attention for query block u, one head.

                The denominator is accumulated OFF the PE: gpsimd adds P
                chunks pairwise into f32 tiles; one f32 ones-matmul per head
                broadcasts the cross-partition sum.  pv matmuls lag two
                S-pairs behind (pend), carrying across the head boundary.
                """
                b, j = divmod(u, NTB_B)
                n_k = 4 * (j + 1)
                pend = []
                for h in hsel:
                    stripes = []
                    for _si in range((n_k + 7) // 8):
                        p_stripe = ppool.tile([128, 4096], BF16, tag="P")
                        stripes.append(p_stripe)

                    def poff(m):
                        return 128 * max(0, m - 4 * j)

                    def pchunk(m, stripes=stripes):
                        o = poff(m)
                        return stripes[m // 8][:, (m % 8) * 512 + o : (m % 8) * 512 + 512]

                    pv = pool_a.tile([128, 512], F32, tag="a", name="pv")
                    den = pool_dv.tile([128, 512], F32, tag="dv", name="den")

                    def denpv_pair(g, h=h, pv=pv, den=den, pchunk=pchunk):
                        for w in (0, 1):
                            m = 2 * g + w
                            o = poff(m)
                            nc.tensor.matmul(
                                den[:, o:512], ones_s, pchunk(m),
                                start=(m == 0), stop=(m == n_k - 1),
                            )
                            nc.tensor.matmul(
                                pv[:, o:512], VH[b][:, m, h * D : (h + 1) * D], pchunk(m),
                                start=(m == 0), stop=(m == n_k - 1),
                            )
                        if 2 * g + 1 == n_k - 1:
                            rc = rcp.tile([128, 512], F32, tag="rc", name="rc")
                            nc.vector.reciprocal_approx_fast(out=rc, in_=den)
                            qsl = slice(j * TBS, (j + 1) * TBS)
                            nc.vector.tensor_mul(YT[b][:, h, qsl], pv, rc)

                    for m in range(n_k):
                        o = poff(m)
                        sg = pool_sg.tile([128, 512], F32, tag="sg", name="sg")
                        nc.tensor.matmul(
                            sg[:, o:512],
                            QKH[b][:, 1, h, m * 128 : (m + 1) * 128],
                            QKH[b][:, 0, h, j * TBS + o : (j + 1) * TBS],
                            start=True,
                            stop=True,
                        )
                        sbase = (m % 8) * 512
                        stripe = stripes[m // 8]
                        nc.scalar.activation(
                            stripe[:, sbase + o : sbase + 512], sg[:, o:512],
                            mybir.ActivationFunctionType.Exp, scale=INV_SQRT_D,
                        )
                        r = m - 4 * j
                        if r >= 0:
                            ck = stripe[
                                :, sbase + 128 * r : sbase + 128 * r + 128
                            ]
                            nc.gpsimd.affine_select(
                                out=ck,
                                in_=ck,
                                compare_op=mybir.AluOpType.is_ge,
                                fill=0.0,
                                base=0,
                                pattern=[[1, 128]],
                                channel_multiplier=-1,
                            )
                        if m % 2 == 1:
                            pend.append((denpv_pair, m // 2))
                            if len(pend) > 4:
                                fn, ga = pend.pop(0)
                                fn(ga)
                return pend

            def stage_c(u, isel):
                """partial c_proj for query block u (bf16 output).

                For the two drain-phase blocks (no stage_a running) the PSUM
                evictions alternate Scalar/Vector to halve the tail latency.
                """
                b, j = divmod(u, NTB_B)
                alt = u >= NU - 2
                for i4 in isel:
                    i = j * 4 + i4
                    row0 = b * T + i * 128
                    ot = outp.tile([128, C], BF16, tag="o")
                    for n2 in range(C // 512):
                        ps = pool_a.tile([128, 512], F32, tag="a", name="cps")
                        for hh in range(HPC):
                            nc.tensor.matmul(
                                ps,
                                YT[b][:, hh, i * 128 : (i + 1) * 128],
                                wp_s[:, hh, n2 * 512 : (n2 + 1) * 512],
                                start=(hh == 0),
                                stop=(hh == HPC - 1),
                            )
                        if alt and n2 % 2 == 0:
                            nc.scalar.activation(
                                ot[:, n2 * 512 : (n2 + 1) * 512], ps,
                                mybir.ActivationFunctionType.Copy,
                            )
                        else:
                            nc.vector.tensor_copy(ot[:, n2 * 512 : (n2 + 1) * 512], ps)
                    nc.sync.dma_start(out_d[row0 : row0 + 128, :], ot)

            def flush(pend):
                while pend:
                    fn, ga = pend.pop(0)
                    fn(ga)

            for step in range(NU + 2):
                if step < NU:
                    stage_a(step)
                if 1 <= step <= NU:
                    flush(stage_b(step - 1, (0,)))
                if step >= 2:
                    stage_c(step - 2, (0, 1))
                if 1 <= step <= NU:
                    flush(stage_b(step - 1, (1,)))
                if step >= 2:
                    stage_c(step - 2, (2, 3))

    nc.compile()
    return nc


def _host_prep(x, w_atten, w_proj):
    """Build the shared + per-core input arrays."""
    x = np.asarray(x, dtype=np.float32)
    w_atten = np.asarray(w_atten, dtype=np.float32)
    w_proj = np.asarray(w_proj, dtype=np.float32)

    # xb[u, p, k, t] = x[token u*512+t, channel k*128+p]
    xf = x.reshape(B * NTB_B, TBS, NKT, 128)
    xb = np.ascontiguousarray(xf.transpose(0, 3, 2, 1)).astype(NP_BF16)

    wq = w_atten[0:C]
    wk = w_atten[C : 2 * C]
    wv_full = w_atten[2 * C : 3 * C]

    # rope tables: theta_i = base^(-2i/D)
    theta = 1.0 / (10000.0 ** (np.arange(0, D, 2, dtype=np.float64) / D))  # [64]
    tpos = np.arange(T, dtype=np.float64)
    ang = np.outer(theta, tpos)  # [64, T]
    cs_half = np.cos(ang).astype(np.float32)
    sn_half = np.sin(ang).astype(np.float32)
    cs = np.concatenate([cs_half, cs_half], axis=0).astype(NP_BF16)  # [128, T]
    sn = np.concatenate([sn_half, sn_half], axis=0).astype(NP_BF16)
    cs = np.ascontiguousarray(cs.reshape(128, 4, 512).transpose(1, 0, 2))
    sn = np.ascontiguousarray(sn.reshape(128, 4, 512).transpose(1, 0, 2))

    ones_f = np.ones((128, 128), dtype=np.float32).astype(NP_BF16)

    top_idx = np.arange(0, D, 2)   # 64
    bot_idx = np.arange(1, D, 2)

    in_maps = []
    for c in range(NC_CORES):
        heads = [HPC * c + h for h in range(HPC)]
        # colgroups: 0 (tops of q), 1 (bots of q), 2/3 same for k
        fb = []
        for wmat in (wq, wk):
            for idx in (top_idx, bot_idx):
                rows = np.concatenate([wmat[hh * D + idx] for hh in heads], axis=0)
                fb.append(rows)  # [128, C]
        w_qk_c = np.concatenate(fb, axis=0)  # [512, C]
        wqk_dev = np.ascontiguousarray(
            w_qk_c.T.reshape(NKT, 128, 4 * 128).astype(NP_BF16)
        )
        w_v_c = np.concatenate([wv_full[hh * D : (hh + 1) * D] for hh in heads], axis=0)
        wv_dev = np.ascontiguousarray(
            w_v_c.T.reshape(NKT, 128, HPC * D).astype(NP_BF16)
        )
        cols = np.concatenate([np.arange(hh * D, (hh + 1) * D) for hh in heads])
        w_p_c = np.ascontiguousarray(w_proj[:, cols].T)  # [256, C]
        wp_dev = np.ascontiguousarray(
            w_p_c.reshape(HPC, 128, C // 512, 512).transpose(0, 2, 1, 3).astype(NP_BF16)
        )
        in_maps.append(
            {
                "xb": xb,
                "wqk": wqk_dev,
                "wv": wv_dev,
                "wp": wp_dev,
                "cs": cs,
                "sn": sn,
                "ones_f": ones_f,
            }
        )
    return in_maps


def _execute(in_maps, trace=False, trace_kwargs=None):
    if "nc" not in _CACHE:
        _CACHE["nc"] = _build_program()
    nc = _CACHE["nc"]
    kwargs = {}
    if trace:
        _install_ntff_hook()
        kwargs["trace"] = True
        if trace_kwargs:
            kwargs.update(trace_kwargs)
    return run_bass_kernel_spmd(nc, in_maps, core_ids=list(range(NC_CORES)), **kwargs)


def _install_ntff_hook():
    """Restore the axon NTFF profile hook (the container's antenv lacks it)."""
    import types

    if "antenv.axon_hooks" in sys.modules:
        return
    mod = types.ModuleType("antenv.axon_hooks")
    mod._hook = None

    def set_axon_ntff_profile_hook(h):
        mod._hook = h

    def get_axon_ntff_profile_hook():
        if mod._hook is None:
            try:
                from trn_agent_boot.trn_boot import _ntff_profile_via_ctypes

                mod._hook = _ntff_profile_via_ctypes("/opt/axon/libaxon_pjrt.so")
            except Exception:
                mod._hook = None
        return mod._hook

    mod.set_axon_ntff_profile_hook = set_axon_ntff_profile_hook
    mod.get_axon_ntff_profile_hook = get_axon_ntff_profile_hook
    sys.modules["antenv.axon_hooks"] = mod


def kernel(x, w_atten, w_proj):
    in_maps = _host_prep(x, w_atten, w_proj)
    res = _execute(in_maps)
    total = res.results[0]["out"].astype(np.float32)
    for c in range(1, NC_CORES):
        total = total + res.results[c]["out"].astype(np.float32)
    return total.reshape(B, T, C)
